# revision 19
# baseline (speedup 1.0000x reference)
"""MoE batched-experts kernel for Trainium2 (8 NeuronCores, expert-parallel).

Problem: out[n] = sum_e routing[n,e] * MLP_e(x[n]) with N=4096, D=1024,
E=16 experts, H=2048, top-2 routing (routing_tensor is zero except for
each token's 2 chosen experts).

Strategy: only the chosen (token, expert) pairs are computed (8x fewer
FLOPs than the dense reference). Experts are sharded 2-per-core (largest
paired with smallest for load balance). The host gathers each expert's
tokens (transposed to [D, T]), the device runs the 2-layer MLP with
fp32r matmuls (full PE rate, ~1e-4 relative rounding) + exact-Gelu on
the scalar engine, scales rows by the routing score, and the host
scatter-adds the per-expert outputs back into the [N, D] result.
"""

import math
import os

import numpy as np

P = 128
NE = 16  # experts
D = 1024
H = 2048

_CACHE: dict = {}
LAST_RESULTS = None  # BassKernelResults of the most recent device run


def _t_chunks(T):
    """Split T into moving-dim chunks <=512; keep every chunk >=256 when
    possible (fp32r matmuls with moving dim <256 run at 1/4 rate)."""
    out = []
    t = 0
    rem = T
    while rem > 512:
        if rem >= 768:
            out.append((t, 512))
            t += 512
            rem -= 512
        else:  # 513..767: split as (rem-256, 256)
            out.append((t, rem - 256))
            t += rem - 256
            rem = 256
    out.append((t, rem))
    return out


def _build(T0: int, T1: int):
    """Build + compile the 2-expert-per-core MLP program.

    T0/T1: token count for expert slot 0 (large) / 1 (small).
    """
    import concourse.mybir as mybir
    import concourse.tile as tile
    from concourse import bacc

    F32 = mybir.dt.float32
    F32R = mybir.dt.float32r
    AF = mybir.ActivationFunctionType

    KD = D // P   # 8 contraction chunks for x @ W0
    KH = H // P   # 16 contraction chunks for h @ W1
    HHALF = H // 2
    B0, B1 = math.ceil(T0 / P), math.ceil(T1 / P)

    nc = bacc.Bacc("TRN2", target_bir_lowering=False, debug=False, num_devices=8)
    # [d_chunk, p, t] gathered tokens, transposed (x^T)
    xt0 = nc.dram_tensor("xt0", [KD, P, T0], F32R, kind="ExternalInput").ap()
    xt1 = nc.dram_tensor("xt1", [KD, P, T1], F32R, kind="ExternalInput").ap()
    # W0 as [half, d_chunk, p, HHALF]; W1 as [h_chunk, p, D]
    w0a = nc.dram_tensor("w0a", [2, KD, P, HHALF], F32R, kind="ExternalInput").ap()
    w0b = nc.dram_tensor("w0b", [2, KD, P, HHALF], F32R, kind="ExternalInput").ap()
    w1a = nc.dram_tensor("w1a", [KH, P, D], F32R, kind="ExternalInput").ap()
    w1b = nc.dram_tensor("w1b", [KH, P, D], F32R, kind="ExternalInput").ap()
    # b0 per slot: [p, 2*KH] (col s*KH+h = b0[h*128:(h+1)*128] of slot s)
    b0s = nc.dram_tensor("b0s", [P, 2 * KH], F32, kind="ExternalInput").ap()
    # routing scores: col j = scores of token block j (slot0 blocks then slot1)
    scl = nc.dram_tensor("scl", [P, B0 + B1], F32, kind="ExternalInput").ap()
    y = nc.dram_tensor("y", [T0 + T1, D], F32, kind="ExternalOutput").ap()

    with tile.TileContext(nc) as tc:
        with tc.tile_pool(name="wp", bufs=1) as wp, \
             tc.tile_pool(name="dp", bufs=1) as dp, \
             tc.tile_pool(name="op", bufs=3) as op, \
             tc.tile_pool(name="ps", bufs=8, space="PSUM") as ps:
            for s, (xt_in, w0_in, w1_in, T, B) in enumerate(
                [(xt0, w0a, w1a, T0, B0), (xt1, w0b, w1b, T1, B1)]
            ):
                # --- input DMAs: first-needed-first, split across queues.
                # gpsimd: xt (+ constants); sync: weights (w0 half0
                # interleaved with xt so the d-accumulation can start after
                # one (xt_d, w0) pair lands).
                xts = []
                w0t = [[None] * KD for _ in range(2)]
                tw0 = _t_chunks(T)[0][1] if s == 0 else T
                for d in range(KD):
                    xt = dp.tile([P, T], F32R, tag=f"xt{d}", name=f"xt_s{s}_d{d}")
                    if tw0 < T:
                        # slot0 startup is HBM-bound: keep only the
                        # first-chunk columns on the critical path (fast
                        # sync queue); the rest streams later via gpsimd
                        nc.sync.dma_start(xt[:, :tw0], xt_in[d, :, :tw0])
                    else:
                        nc.gpsimd.dma_start(xt[:], xt_in[d])
                    xts.append(xt)
                    w = wp.tile([P, HHALF], F32R, tag=f"w0_0_{d}",
                                name=f"w0_s{s}_0_{d}")
                    nc.sync.dma_start(w[:], w0_in[0, d])
                    w0t[0][d] = w
                if s == 0:
                    b0t = dp.tile([P, 2 * KH], F32, tag="b0t")
                    nc.gpsimd.dma_start(b0t[:], b0s)
                    sclt = dp.tile([P, B0 + B1], F32, tag="sclt")
                    nc.gpsimd.dma_start(sclt[:], scl)
                if tw0 < T:
                    for d in range(KD):
                        nc.gpsimd.dma_start(xts[d][:, tw0:], xt_in[d, :, tw0:])
                for d in range(KD):
                    w = wp.tile([P, HHALF], F32R, tag=f"w0_1_{d}",
                                name=f"w0_s{s}_1_{d}")
                    nc.sync.dma_start(w[:], w0_in[1, d])
                    w0t[1][d] = w
                w1t = []
                for h in range(KH):
                    w = wp.tile([P, D], F32R, tag=f"w1_{h}", name=f"w1_s{s}_{h}")
                    nc.sync.dma_start(w[:], w1_in[h])
                    w1t.append(w)
                hts = []
                for h in range(KH):
                    ht = dp.tile([P, T], F32R, tag=f"ht{h}", name=f"ht_s{s}_{h}")
                    hts.append(ht)

                # --- stage 1: hT[h, t] = gelu(x @ W0 + b0), h on partitions.
                # half-outer so W0-half0 is fully consumed (both t-chunks)
                # before half1 is needed -> streams at sustainable DMA rate.
                for half in range(2):
                    for (t0, tw) in _t_chunks(T):
                        pt = []
                        for hh in range(KH // 2):
                            p_ = ps.tile([P, 512], F32, tag="ps",
                                         name=f"ps1_s{s}_{half}_{t0}_{hh}")
                            pt.append(p_)
                        for d in range(KD):
                            for hh in range(KH // 2):
                                nc.tensor.matmul(
                                    pt[hh][:, :tw],
                                    w0t[half][d][:, hh * P:(hh + 1) * P],
                                    xts[d][:, t0:t0 + tw],
                                    start=(d == 0), stop=(d == KD - 1),
                                )
                        for hh in range(KH // 2):
                            h = half * (KH // 2) + hh
                            nc.scalar.activation(
                                hts[h][:, t0:t0 + tw], pt[hh][:, :tw],
                                AF.Gelu, bias=b0t[:, s * KH + h: s * KH + h + 1],
                            )

                # --- stage 2: y[t, d] = s_t * (hT.T @ W1) ---
                for tt in range(B):
                    tsz = min(P, T - tt * P)
                    ot = op.tile([P, D], F32, tag="ot", name=f"ot_s{s}_{tt}")
                    for dc in range(2):
                        p2 = ps.tile([P, 512], F32, tag="ps",
                                     name=f"ps2_s{s}_{tt}_{dc}")
                        for h in range(KH):
                            nc.tensor.matmul(
                                p2[:tsz, :],
                                hts[h][:, tt * P: tt * P + tsz],
                                w1t[h][:, dc * 512:(dc + 1) * 512],
                                start=(h == 0), stop=(h == KH - 1),
                            )
                        scol = (0 if s == 0 else B0) + tt
                        nc.vector.tensor_scalar_mul(
                            ot[:tsz, dc * 512:(dc + 1) * 512], p2[:tsz, :],
                            sclt[:tsz, scol:scol + 1],
                        )
                    row0 = (0 if s == 0 else T0) + tt * P
                    nc.sync.dma_start(y[row0:row0 + tsz, :], ot[:tsz, :])

    nc.compile()
    return nc


def _ensure_ntff_hook_module():
    """bass_utils unconditionally imports antenv.axon_hooks when tracing is
    requested; on images without it, provide a shim wired to the axon
    ctypes profiler when available (else a no-hook fallback)."""
    import importlib.util
    import sys
    import types

    if importlib.util.find_spec("antenv") is None:
        return
    try:
        import antenv.axon_hooks  # noqa: F401
        return
    except ImportError:
        pass
    mod = types.ModuleType("antenv.axon_hooks")
    mod._hook = None

    def set_axon_ntff_profile_hook(h):
        mod._hook = h

    def get_axon_ntff_profile_hook():
        return mod._hook

    mod.set_axon_ntff_profile_hook = set_axon_ntff_profile_hook
    mod.get_axon_ntff_profile_hook = get_axon_ntff_profile_hook
    try:
        from trn_agent_boot.trn_boot import _ntff_profile_via_ctypes
        mod._hook = _ntff_profile_via_ctypes("/opt/axon/libaxon_pjrt.so")
    except Exception:
        pass
    sys.modules["antenv.axon_hooks"] = mod
    import antenv
    antenv.axon_hooks = mod


def kernel(x, routing_tensor, W0, b0, W1, b1):
    global LAST_RESULTS
    from concourse.bass_utils import run_bass_kernel_spmd
    _ensure_ntff_hook_module()

    x = np.ascontiguousarray(x, dtype=np.float32)
    routing = np.asarray(routing_tensor, dtype=np.float32)
    W0 = np.asarray(W0, dtype=np.float32)
    b0 = np.asarray(b0, dtype=np.float32)
    W1 = np.asarray(W1, dtype=np.float32)
    b1 = np.asarray(b1, dtype=np.float32)

    # --- routing prep: per-expert token lists ---
    idx = [np.nonzero(routing[:, e])[0] for e in range(NE)]
    counts = np.array([len(i) for i in idx])
    order = np.argsort(-counts, kind="stable")  # experts sorted by load desc
    big, small = order[:8], order[8:][::-1]     # pair rank i with rank 15-i
    # fp32r matmuls require an even moving dim -> round T up to even
    T0 = max(P, int(counts[big].max()) + 1 >> 1 << 1)
    T1 = max(P, int(counts[small].max()) + 1 >> 1 << 1)
    B0, B1 = math.ceil(T0 / P), math.ceil(T1 / P)

    key = (T0, T1)
    if key not in _CACHE:
        _CACHE[key] = _build(T0, T1)
    nc = _CACHE[key]

    # --- build per-core inputs ---
    in_maps = []
    KD, KH, HHALF = D // P, H // P, H // 2
    for c in range(8):
        ea, eb = int(big[c]), int(small[c])
        m = {}
        for name, e, T in (("xt0", ea, T0), ("xt1", eb, T1)):
            g = np.zeros((T, D), np.float32)
            g[: len(idx[e])] = x[idx[e]]
            m[name] = np.ascontiguousarray(g.T.reshape(KD, P, T))
        m["w0a"] = np.ascontiguousarray(
            W0[ea].reshape(KD, P, 2, HHALF).transpose(2, 0, 1, 3))
        m["w0b"] = np.ascontiguousarray(
            W0[eb].reshape(KD, P, 2, HHALF).transpose(2, 0, 1, 3))
        m["w1a"] = np.ascontiguousarray(W1[ea].reshape(KH, P, D))
        m["w1b"] = np.ascontiguousarray(W1[eb].reshape(KH, P, D))
        b0m = np.zeros((P, 2 * KH), np.float32)
        b0m[:, :KH] = b0[ea].reshape(KH, P).T
        b0m[:, KH:] = b0[eb].reshape(KH, P).T
        m["b0s"] = b0m
        sa = routing[idx[ea], ea]
        sb = routing[idx[eb], eb]
        sc = np.zeros(((B0 + B1) * P,), np.float32)
        sc[: len(sa)] = sa
        sc[B0 * P: B0 * P + len(sb)] = sb
        m["scl"] = np.ascontiguousarray(sc.reshape(B0 + B1, P).T)
        in_maps.append(m)

    res = run_bass_kernel_spmd(nc, in_maps, core_ids=list(range(8)),
                               trace=bool(os.environ.get("BASS_TRACE")))
    LAST_RESULTS = res

    # --- combine: out = routing @ b1 + scatter-add of per-expert rows ---
    out = routing @ b1
    for c in range(8):
        yc = res.results[c]["y"]
        ea, eb = int(big[c]), int(small[c])
        out[idx[ea]] += yc[: len(idx[ea])]
        out[idx[eb]] += yc[T0: T0 + len(idx[eb])]
    return out.astype(np.float32)


# revision 20
# speedup vs baseline: 1.0251x; 1.0251x over previous
"""MoE batched-experts kernel for Trainium2 (8 NeuronCores, expert-parallel).

Problem: out[n] = sum_e routing[n,e] * MLP_e(x[n]) with N=4096, D=1024,
E=16 experts, H=2048, top-2 routing (routing_tensor is zero except for
each token's 2 chosen experts).

Strategy: only the chosen (token, expert) pairs are computed (8x fewer
FLOPs than the dense reference). Experts are sharded 2-per-core (largest
paired with smallest for load balance). The host gathers each expert's
tokens (transposed to [D, T]), the device runs the 2-layer MLP with
fp32r matmuls (full PE rate, ~1e-4 relative rounding) + exact-Gelu on
the scalar engine, scales rows by the routing score, and the host
scatter-adds the per-expert outputs back into the [N, D] result.
"""

import math
import os

import numpy as np

P = 128
NE = 16  # experts
D = 1024
H = 2048

_CACHE: dict = {}
LAST_RESULTS = None  # BassKernelResults of the most recent device run


def _t_chunks(T):
    """Split T into moving-dim chunks <=512; keep every chunk >=256 when
    possible (fp32r matmuls with moving dim <256 run at 1/4 rate)."""
    out = []
    t = 0
    rem = T
    while rem > 512:
        if rem >= 768:
            out.append((t, 512))
            t += 512
            rem -= 512
        else:  # 513..767: split as (rem-256, 256)
            out.append((t, rem - 256))
            t += rem - 256
            rem = 256
    out.append((t, rem))
    return out


def _build(T0: int, T1: int):
    """Build + compile the 2-expert-per-core MLP program.

    T0/T1: token count for expert slot 0 (large) / 1 (small).
    """
    import concourse.mybir as mybir
    import concourse.tile as tile
    from concourse import bacc

    F32 = mybir.dt.float32
    F32R = mybir.dt.float32r
    AF = mybir.ActivationFunctionType

    KD = D // P   # 8 contraction chunks for x @ W0
    KH = H // P   # 16 contraction chunks for h @ W1
    HHALF = H // 2
    B0, B1 = math.ceil(T0 / P), math.ceil(T1 / P)

    nc = bacc.Bacc("TRN2", target_bir_lowering=False, debug=False, num_devices=8)
    # [d_chunk, p, t] gathered tokens, transposed (x^T)
    xt0 = nc.dram_tensor("xt0", [KD, P, T0], F32R, kind="ExternalInput").ap()
    xt1 = nc.dram_tensor("xt1", [KD, P, T1], F32R, kind="ExternalInput").ap()
    # W0 as [half, d_chunk, p, HHALF]; W1 as [h_chunk, p, D]
    w0a = nc.dram_tensor("w0a", [2, KD, P, HHALF], F32R, kind="ExternalInput").ap()
    w0b = nc.dram_tensor("w0b", [2, KD, P, HHALF], F32R, kind="ExternalInput").ap()
    w1a = nc.dram_tensor("w1a", [KH, P, D], F32R, kind="ExternalInput").ap()
    w1b = nc.dram_tensor("w1b", [KH, P, D], F32R, kind="ExternalInput").ap()
    # b0 per slot: [p, 2*KH] (col s*KH+h = b0[h*128:(h+1)*128] of slot s)
    b0s = nc.dram_tensor("b0s", [P, 2 * KH], F32, kind="ExternalInput").ap()
    # routing scores: col j = scores of token block j (slot0 blocks then slot1)
    scl = nc.dram_tensor("scl", [P, B0 + B1], F32, kind="ExternalInput").ap()
    y = nc.dram_tensor("y", [T0 + T1, D], F32, kind="ExternalOutput").ap()

    with tile.TileContext(nc) as tc:
        with tc.tile_pool(name="wp", bufs=1) as wp, \
             tc.tile_pool(name="dp", bufs=1) as dp, \
             tc.tile_pool(name="op", bufs=3) as op, \
             tc.tile_pool(name="ps", bufs=8, space="PSUM") as ps:
            for s, (xt_in, w0_in, w1_in, T, B) in enumerate(
                [(xt0, w0a, w1a, T0, B0), (xt1, w0b, w1b, T1, B1)]
            ):
                # --- input DMAs: first-needed-first, split across queues.
                # gpsimd: xt (+ constants); sync: weights (w0 half0
                # interleaved with xt so the d-accumulation can start after
                # one (xt_d, w0) pair lands).
                xts = []
                w0t = [[None] * KD for _ in range(2)]
                tw0 = _t_chunks(T)[0][1] if s == 0 else T
                for d in range(KD):
                    xt = dp.tile([P, T], F32R, tag=f"xt{d}", name=f"xt_s{s}_d{d}")
                    if tw0 < T:
                        # slot0 startup is HBM-bound: keep only the
                        # first-chunk columns on the critical path (fast
                        # sync queue); the rest streams later via gpsimd
                        nc.sync.dma_start(xt[:, :tw0], xt_in[d, :, :tw0])
                    else:
                        nc.gpsimd.dma_start(xt[:], xt_in[d])
                    xts.append(xt)
                    w = wp.tile([P, HHALF], F32R, tag=f"w0_0_{d}",
                                name=f"w0_s{s}_0_{d}")
                    nc.sync.dma_start(w[:], w0_in[0, d])
                    w0t[0][d] = w
                if s == 0:
                    b0t = dp.tile([P, 2 * KH], F32, tag="b0t")
                    nc.gpsimd.dma_start(b0t[:], b0s)
                    sclt = dp.tile([P, B0 + B1], F32, tag="sclt")
                    nc.gpsimd.dma_start(sclt[:], scl)
                if tw0 < T:
                    for d in range(KD):
                        nc.gpsimd.dma_start(xts[d][:, tw0:], xt_in[d, :, tw0:])
                for d in range(KD):
                    w = wp.tile([P, HHALF], F32R, tag=f"w0_1_{d}",
                                name=f"w0_s{s}_1_{d}")
                    nc.sync.dma_start(w[:], w0_in[1, d])
                    w0t[1][d] = w
                w1t = []
                for h in range(KH):
                    w = wp.tile([P, D], F32R, tag=f"w1_{h}", name=f"w1_s{s}_{h}")
                    nc.sync.dma_start(w[:], w1_in[h])
                    w1t.append(w)
                hts = []
                for h in range(KH):
                    ht = dp.tile([P, T], F32R, tag=f"ht{h}", name=f"ht_s{s}_{h}")
                    hts.append(ht)

                # --- stage 1: hT[h, t] = gelu(x @ W0 + b0), h on partitions.
                # half-outer so W0-half0 is fully consumed (both t-chunks)
                # before half1 is needed -> streams at sustainable DMA rate.
                for half in range(2):
                    for (t0, tw) in _t_chunks(T):
                        pt = []
                        for hh in range(KH // 2):
                            p_ = ps.tile([P, 512], F32, tag="ps",
                                         name=f"ps1_s{s}_{half}_{t0}_{hh}")
                            pt.append(p_)
                        for d in range(KD):
                            for hh in range(KH // 2):
                                nc.tensor.matmul(
                                    pt[hh][:, :tw],
                                    w0t[half][d][:, hh * P:(hh + 1) * P],
                                    xts[d][:, t0:t0 + tw],
                                    start=(d == 0), stop=(d == KD - 1),
                                )
                        for hh in range(KH // 2):
                            h = half * (KH // 2) + hh
                            nc.scalar.activation(
                                hts[h][:, t0:t0 + tw], pt[hh][:, :tw],
                                AF.Gelu, bias=b0t[:, s * KH + h: s * KH + h + 1],
                            )

                # --- stage 2: y[t, d] = s_t * (hT.T @ W1) ---
                for tt in range(B):
                    tsz = min(P, T - tt * P)
                    ot = op.tile([P, D], F32, tag="ot", name=f"ot_s{s}_{tt}")
                    for dc in range(2):
                        p2 = ps.tile([P, 512], F32, tag="ps",
                                     name=f"ps2_s{s}_{tt}_{dc}")
                        for h in range(KH):
                            nc.tensor.matmul(
                                p2[:tsz, :],
                                hts[h][:, tt * P: tt * P + tsz],
                                w1t[h][:, dc * 512:(dc + 1) * 512],
                                start=(h == 0), stop=(h == KH - 1),
                            )
                        scol = (0 if s == 0 else B0) + tt
                        nc.scalar.activation(
                            ot[:tsz, dc * 512:(dc + 1) * 512], p2[:tsz, :],
                            AF.Copy, scale=sclt[:tsz, scol:scol + 1],
                        )
                    row0 = (0 if s == 0 else T0) + tt * P
                    nc.sync.dma_start(y[row0:row0 + tsz, :], ot[:tsz, :])

    nc.compile()
    return nc


def _ensure_ntff_hook_module():
    """bass_utils unconditionally imports antenv.axon_hooks when tracing is
    requested; on images without it, provide a shim wired to the axon
    ctypes profiler when available (else a no-hook fallback)."""
    import importlib.util
    import sys
    import types

    if importlib.util.find_spec("antenv") is None:
        return
    try:
        import antenv.axon_hooks  # noqa: F401
        return
    except ImportError:
        pass
    mod = types.ModuleType("antenv.axon_hooks")
    mod._hook = None

    def set_axon_ntff_profile_hook(h):
        mod._hook = h

    def get_axon_ntff_profile_hook():
        return mod._hook

    mod.set_axon_ntff_profile_hook = set_axon_ntff_profile_hook
    mod.get_axon_ntff_profile_hook = get_axon_ntff_profile_hook
    try:
        from trn_agent_boot.trn_boot import _ntff_profile_via_ctypes
        mod._hook = _ntff_profile_via_ctypes("/opt/axon/libaxon_pjrt.so")
    except Exception:
        pass
    sys.modules["antenv.axon_hooks"] = mod
    import antenv
    antenv.axon_hooks = mod


def kernel(x, routing_tensor, W0, b0, W1, b1):
    global LAST_RESULTS
    from concourse.bass_utils import run_bass_kernel_spmd
    _ensure_ntff_hook_module()

    x = np.ascontiguousarray(x, dtype=np.float32)
    routing = np.asarray(routing_tensor, dtype=np.float32)
    W0 = np.asarray(W0, dtype=np.float32)
    b0 = np.asarray(b0, dtype=np.float32)
    W1 = np.asarray(W1, dtype=np.float32)
    b1 = np.asarray(b1, dtype=np.float32)

    # --- routing prep: per-expert token lists ---
    idx = [np.nonzero(routing[:, e])[0] for e in range(NE)]
    counts = np.array([len(i) for i in idx])
    order = np.argsort(-counts, kind="stable")  # experts sorted by load desc
    big, small = order[:8], order[8:][::-1]     # pair rank i with rank 15-i
    # fp32r matmuls require an even moving dim -> round T up to even
    T0 = max(P, int(counts[big].max()) + 1 >> 1 << 1)
    T1 = max(P, int(counts[small].max()) + 1 >> 1 << 1)
    B0, B1 = math.ceil(T0 / P), math.ceil(T1 / P)

    key = (T0, T1)
    if key not in _CACHE:
        _CACHE[key] = _build(T0, T1)
    nc = _CACHE[key]

    # --- build per-core inputs ---
    in_maps = []
    KD, KH, HHALF = D // P, H // P, H // 2
    for c in range(8):
        ea, eb = int(big[c]), int(small[c])
        m = {}
        for name, e, T in (("xt0", ea, T0), ("xt1", eb, T1)):
            g = np.zeros((T, D), np.float32)
            g[: len(idx[e])] = x[idx[e]]
            m[name] = np.ascontiguousarray(g.T.reshape(KD, P, T))
        m["w0a"] = np.ascontiguousarray(
            W0[ea].reshape(KD, P, 2, HHALF).transpose(2, 0, 1, 3))
        m["w0b"] = np.ascontiguousarray(
            W0[eb].reshape(KD, P, 2, HHALF).transpose(2, 0, 1, 3))
        m["w1a"] = np.ascontiguousarray(W1[ea].reshape(KH, P, D))
        m["w1b"] = np.ascontiguousarray(W1[eb].reshape(KH, P, D))
        b0m = np.zeros((P, 2 * KH), np.float32)
        b0m[:, :KH] = b0[ea].reshape(KH, P).T
        b0m[:, KH:] = b0[eb].reshape(KH, P).T
        m["b0s"] = b0m
        sa = routing[idx[ea], ea]
        sb = routing[idx[eb], eb]
        sc = np.zeros(((B0 + B1) * P,), np.float32)
        sc[: len(sa)] = sa
        sc[B0 * P: B0 * P + len(sb)] = sb
        m["scl"] = np.ascontiguousarray(sc.reshape(B0 + B1, P).T)
        in_maps.append(m)

    res = run_bass_kernel_spmd(nc, in_maps, core_ids=list(range(8)),
                               trace=bool(os.environ.get("BASS_TRACE")))
    LAST_RESULTS = res

    # --- combine: out = routing @ b1 + scatter-add of per-expert rows ---
    out = routing @ b1
    for c in range(8):
        yc = res.results[c]["y"]
        ea, eb = int(big[c]), int(small[c])
        out[idx[ea]] += yc[: len(idx[ea])]
        out[idx[eb]] += yc[T0: T0 + len(idx[eb])]
    return out.astype(np.float32)
